# revision 80
# baseline (speedup 1.0000x reference)
"""Trainium2 kernel v8: per-tile dtype-mix balanced streaming max-pool.

out[n] = (1/32768) * sum_{c,windows} maxpool3d_2x2x2(o[n]) + sum_c bias[c]

Cost-model facts driving the design (per core, per 128-row tile of 8192
values/partition):
  - DMA: 360 B/ns aggregate (exclusive DMA_ENGINES device). fp8 chunk =
    256 B, bf16 chunk = 512 B per partition per 256-value chunk; a DMA
    completion's semaphore fires 900 ns after the transfer ends.
  - DVE tensor_max: 2 elem/cycle @0.96 GHz only when every operand is a
    packed 2-byte dtype; fp8 operands run 1x. The 7-op max tree per
    8-value window costs 3.65 ns/window (bf16) -> ~30 us/core minimum,
    which makes DVE the pipeline spine.
  - ACT: any unary 1 elem/cycle @1.2 GHz (fp8->bf16 upcast = 213 ns per
    256-value chunk) + ~185 ns fixed per instruction -> at most ~18
    upcast chunks/tile can sustain a one-tile-ahead ACT pipeline.
  - PE matmul: partition-dim sums are free -> all summation rides PE
    into PSUM banks (one bank per batch, region-matmuls for tile 7).

Per tile, NB chunks ship bf16 (DVE L1 reads the DMA data directly), NAL
chunks ship fp8 and are upcast by ACT one tile ahead, NGAM chunks ship
fp8 through DVE's 1x L1 (no ACT dependency). Tile 0 is almost all
NGAM: its split xf DMA feeds DVE from ~3.8 us with zero ACT coupling,
while xf1 is prefetched mid-tile-0 so ACT's upcast pipeline starts
immediately after. Mid tiles keep one small NGAM chunk as slack that
absorbs ACT drift. Tile 7 is bf16-heavy and processed in 4 pieces: two
feed a PSUM bank closed out during the stream, the last two end in
fused L3+row-sum (stt) accumulators whose [1,1] matmuls land directly
in the PSUM result row, so the post-stream tail is one short DVE chain
plus the output store.
"""

import numpy as np

N, C, D, H, W = 16, 32, 32, 64, 64
N_CORES = 8
N_PER_CORE = N // N_CORES          # 2
PD = D // 2                        # 16
ROWS = N_PER_CORE * C * PD         # 1024 rows/core, n-major
P = 128
N_TILES = ROWS // P                # 8
TILES_PER_N = N_TILES // N_PER_CORE  # 4

NG = 32                            # 256-value chunks per row
GW = 256
SCALE = 1.0 / (2.0 * PD * (H // 2) * (W // 2))  # 1/32768, exact in bf16

# per-tile (NB bf16, NAL fp8-upcast, NGAM fp8-direct); sums to NG
MIX = [
    (2, 0, 30),
    (14, 15, 3),
    (13, 17, 2),
    (13, 19, 0),
    (13, 18, 1),
    (13, 18, 1),
    (14, 17, 1),
    (20, 12, 0),
]
T7_XB = (8, 8, 4)     # tile 7 bf16 DMA pieces
T0_XF = (10, 10, 5, 5)  # tile 0 fp8 DMA split (fast DVE start)

_NC_CACHE = None


def _build_nc():
    import concourse.bacc as bacc
    import concourse.tile as tile
    import concourse.mybir as mybir

    f32 = mybir.dt.float32
    bf16 = mybir.dt.bfloat16
    f8 = mybir.dt.float8e4
    COPY = mybir.ActivationFunctionType.Copy
    nc = bacc.Bacc("TRN2", target_bir_lowering=False, debug=False)

    tot_f8 = P * GW * sum(m[1] + m[2] for m in MIX)
    tot_bf = P * GW * sum(m[0] for m in MIX)
    xf_in = nc.dram_tensor("xf", [1, tot_f8], f8, kind="ExternalInput")
    xb_in = nc.dram_tensor("xb", [1, tot_bf], bf16, kind="ExternalInput")
    b_in = nc.dram_tensor("bias", [1, C], f32, kind="ExternalInput")
    out_d = nc.dram_tensor("out", [1, N_PER_CORE], f32, kind="ExternalOutput")

    f8_off = [0]
    bf_off = [0]
    for m in MIX:
        f8_off.append(f8_off[-1] + P * GW * (m[1] + m[2]))
        bf_off.append(bf_off[-1] + P * GW * m[0])

    def xf_ap(t):
        nf = MIX[t][1] + MIX[t][2]
        base = f8_off[t]
        return xf_in[:, base : base + P * nf * GW].rearrange(
            "a (p c) -> (a p) c", c=nf * GW
        )

    def xb_ap(t, c0, c1):
        nb = MIX[t][0]
        base = bf_off[t]
        v = xb_in[:, base : base + P * nb * GW].rearrange(
            "a (p c) -> (a p) c", c=nb * GW
        )
        return v[:, c0 * GW : c1 * GW]

    with tile.TileContext(nc) as tc:
        with (
            tc.tile_pool(name="xf", bufs=3) as xfp,
            tc.tile_pool(name="xb", bufs=2) as xbp,
            tc.tile_pool(name="xc", bufs=2) as xcp,
            tc.tile_pool(name="m1", bufs=2) as m1p,
            tc.tile_pool(name="m2", bufs=2) as m2p,
            tc.tile_pool(name="m3", bufs=2) as m3p,
            tc.tile_pool(name="misc", bufs=1) as misc,
            tc.tile_pool(name="ps", bufs=1, space="PSUM") as pp,
        ):
            ones = misc.tile([P, 1], bf16)
            nc.vector.memset(ones[:], SCALE)
            onesf = misc.tile([P, 1], f32)
            nc.vector.memset(onesf[:], SCALE)
            bt = misc.tile([1, C], f32)
            nc.scalar.dma_start(bt[:], b_in[:])
            # partial rows: batch0 [bank, bias]; batch1 [bank, A, B, stt, bias]
            # r1 lives in PSUM so the accT matmul can write col 3 directly.
            r0 = misc.tile([1, 2], f32)
            r1 = pp.tile([1, 4], f32, name="r1", tag="r1")
            bscr = misc.tile([1, C], f32)
            # bias sums on ACT before the first xf lands (ACT is idle then)
            nc.scalar.activation(bscr[:], bt[:], COPY, accum_out=r0[:, 1:2])
            nc.scalar.activation(bscr[:], bt[:], COPY, accum_out=r1[:, 3:4])

            ps = [pp.tile([1, 512], f32, name=f"ps{ni}", tag=f"ps{ni}")
                  for ni in range(N_PER_CORE)]
            accT = misc.tile([P, 2], f32)
            m3s = misc.tile([P, 640], bf16)
            fin = misc.tile([1, N_PER_CORE], f32)
            scr0 = misc.tile([1, 512], f32)
            scr1 = misc.tile([1, 512], f32)

            def fetch_xf(t):
                nf = MIX[t][1] + MIX[t][2]
                if nf == 0:
                    return None
                xft = xfp.tile([P, nf * GW], f8, tag="xf")
                nc.sync.dma_start(xft[:], xf_ap(t))
                return xft

            def fetch_xf0():
                nf = MIX[0][1] + MIX[0][2]
                xft = xfp.tile([P, nf * GW], f8, tag="xf", name="xf0t")
                xb0 = xbp.tile([P, MIX[0][0] * GW], bf16, tag="xb", name="xb0t")
                assert sum(T0_XF) == nf
                xf1 = None
                c0 = 0
                for i, sz in enumerate(T0_XF):
                    nc.sync.dma_start(
                        xft[:, c0 * GW : (c0 + sz) * GW],
                        xf_ap(0)[:, c0 * GW : (c0 + sz) * GW],
                    )
                    c0 += sz
                    if i == 1:
                        xf1 = fetch_xf(1)  # early: ACT's first upcast input
                nc.sync.dma_start(xb0[:], xb_ap(0, 0, MIX[0][0]))
                return xft, xb0, xf1

            def l2l3(m1t, g0, nch, m3t, d0):
                m2t = m2p.tile([P, nch * 64], bf16, tag=f"m2_{g0}_{nch}")
                m1h = m1t[:, g0 * 128 : (g0 + nch) * 128].rearrange(
                    "p (g h w) -> p g h w", h=2, w=64
                )
                nc.vector.tensor_max(
                    m2t[:].rearrange("p (g w) -> p g w", w=64),
                    m1h[:, :, 0, :], m1h[:, :, 1, :],
                )
                m2h = m2t[:].rearrange("p (g h w) -> p g h w", h=2, w=32)
                nc.vector.tensor_max(
                    m3t[:, d0 : d0 + nch * 32].rearrange("p (g w) -> p g w", w=32),
                    m2h[:, :, 0, :], m2h[:, :, 1, :],
                )
                return m2t

            xft_cur, xb0, xft_next = fetch_xf0()
            for t in range(N_TILES - 1):
                nb, nal, ngam = MIX[t]
                n_idx = t // TILES_PER_N
                xft = xft_cur
                xft_cur = xft_next

                if t == 0:
                    xbt = xb0
                else:
                    xbt = xbp.tile([P, nb * GW], bf16, tag="xb")
                    nc.sync.dma_start(xbt[:], xb_ap(t, 0, nb))
                xft_next = (fetch_xf(t + 2) if t + 2 < N_TILES else None)

                if nal:
                    xc = xcp.tile([P, nal * GW], bf16, tag="xc")
                    nc.scalar.activation(xc[:], xft[:, 0 : nal * GW], COPY)

                m1 = m1p.tile([P, NG * 128], bf16, tag="m1")
                m1v = m1[:].rearrange("p (g w) -> p g w", w=128)
                if ngam:
                    # gamma first: it only needs xf (already landed); tile 0's
                    # gamma L1 is split to chase the split xf0 DMAs
                    vg = xft[:, nal * GW :].rearrange("p (g w) -> p g w", w=GW)
                    if t == 0:
                        g0 = 0
                        for sz in T0_XF:
                            nc.vector.tensor_max(
                                m1v[:, nb + nal + g0 : nb + nal + g0 + sz, :],
                                vg[:, g0 : g0 + sz, 0:128],
                                vg[:, g0 : g0 + sz, 128:256],
                            )
                            g0 += sz
                    else:
                        nc.vector.tensor_max(
                            m1v[:, nb + nal :, :], vg[:, :, 0:128], vg[:, :, 128:256]
                        )
                v = xbt[:].rearrange("p (g w) -> p g w", w=GW)
                nc.vector.tensor_max(
                    m1v[:, 0:nb, :], v[:, :, 0:128], v[:, :, 128:256]
                )
                if nal:
                    vc = xc[:].rearrange("p (g w) -> p g w", w=GW)
                    nc.vector.tensor_max(
                        m1v[:, nb : nb + nal, :], vc[:, :, 0:128], vc[:, :, 128:256]
                    )

                m3 = m3p.tile([P, NG * 32], bf16, tag="m3")
                l2l3(m1, 0, NG, m3, 0)

                start = t % TILES_PER_N == 0
                stop = t == 3
                nc.tensor.matmul(ps[n_idx][:], ones[:], m3[:, 0:512],
                                 start=start, stop=False)
                nc.tensor.matmul(ps[n_idx][:], ones[:], m3[:, 512:1024],
                                 start=False, stop=stop)

                if t == 5:
                    # batch 0 finish: data ready since tile 3, off critical path
                    nc.scalar.activation(scr0[:], ps[0][:], COPY,
                                         accum_out=r0[:, 0:1])
                    nc.vector.reduce_sum(fin[:, 0:1], r0[:],
                                         axis=mybir.AxisListType.X)
                    nc.gpsimd.dma_start(out_d[:, 0:1], fin[:, 0:1])

            # ---- tile 7: bf16-heavy, processed in 4 pieces ----
            t = N_TILES - 1
            nb, nal, ngam = MIX[t]
            assert ngam == 0 and sum(T7_XB) == nb
            xft = xft_cur
            xb_pieces = []
            c0 = 0
            for sz in T7_XB:
                xbt = xbp.tile([P, sz * GW], bf16, tag=f"xb7_{c0}")
                nc.sync.dma_start(xbt[:], xb_ap(t, c0, c0 + sz))
                xb_pieces.append(xbt)
                c0 += sz
            xc7 = xcp.tile([P, nal * GW], bf16, tag="xc7")
            nc.scalar.activation(xc7[:], xft[:], COPY)

            # order: a, b -> psA; c, xc7 -> fused L2+stt accumulator
            pieces = [(xb_pieces[0], T7_XB[0]), (xb_pieces[1], T7_XB[1]),
                      (xb_pieces[2], T7_XB[2]), (xc7, nal)]
            m1_7 = m1p.tile([P, NG * 128], bf16, tag="m1_7")
            m1v7 = m1_7[:].rearrange("p (g w) -> p g w", w=128)
            m3_7 = m3p.tile([P, 16 * 32], bf16, tag="m3_7")
            g0 = 0
            for pi, (src, nch) in enumerate(pieces):
                sv = src[:].rearrange("p (g w) -> p g w", w=GW)
                nc.vector.tensor_max(
                    m1v7[:, g0 : g0 + nch, :], sv[:, :, 0:128], sv[:, :, 128:256]
                )
                if pi < 2:
                    with tc.high_priority(offset=40):
                        l2l3(m1_7, g0, nch, m3_7, g0 * 32)
                g0 += nch
                if g0 == 16:
                    with tc.high_priority(offset=40):
                        nc.tensor.matmul(ps[1][:], ones[:], m3_7[:],
                                         start=False, stop=True)
                        # single batch-1 bank reduce covers t4..t7ab
                        nc.scalar.activation(scr1[:], ps[1][:], COPY,
                                             accum_out=r1[:, 0:1])
            # pieces c and xc7: per-piece L2 + fused L3+row-sum (stt)
            for si, (gg0, gg1) in enumerate(((20, 32), (16, 20))):
                nch = gg1 - gg0
                m2s = m2p.tile([P, nch * 64], bf16, tag=f"m2s{si}")
                m1h = m1_7[:, gg0 * 128 : gg1 * 128].rearrange(
                    "p (g h w) -> p g h w", h=2, w=64)
                nc.vector.tensor_max(
                    m2s[:].rearrange("p (g w) -> p g w", w=64),
                    m1h[:, :, 0, :], m1h[:, :, 1, :],
                )
                m2h = m2s[:].rearrange("p (g h w) -> p g h w", h=2, w=32)
                nc.vector.scalar_tensor_tensor(
                    out=m3s[:, si * 384 : si * 384 + nch * 32].rearrange(
                        "p (g w) -> p g w", w=32),
                    in0=m2h[:, :, 0, :],
                    scalar=0.0,
                    in1=m2h[:, :, 1, :],
                    op0=mybir.AluOpType.bypass,
                    op1=mybir.AluOpType.max,
                    accum_out=accT[:, si : si + 1],
                )
            # accT cols -> r1 cols 1,2 via PE (independent regions)
            nc.tensor.matmul(r1[:, 1:2], onesf[:, 0:1],
                             accT[:, 0:1], start=True, stop=True)
            nc.tensor.matmul(r1[:, 2:3], onesf[:, 0:1],
                             accT[:, 1:2], start=True, stop=True)
            nc.vector.reduce_sum(fin[:, 1:2], r1[:], axis=mybir.AxisListType.X)
            nc.sync.dma_start(out_d[:, 1:2], fin[:, 1:2])

    nc.compile()
    return nc


_RUNNER_CACHE = None


def _build_runner(nc):
    """Jitted shard_map runner built once; per call only input upload +
    execution happen."""
    import jax
    import numpy as _np
    from jax.sharding import Mesh, PartitionSpec, NamedSharding
    from concourse import bass2jax
    import concourse.mybir as mybir

    bass2jax.install_neuronx_cc_hook()
    partition_name = nc.partition_id_tensor.name if nc.partition_id_tensor else None
    in_names, out_names, out_avals, zero_outs = [], [], [], []
    for alloc in nc.m.functions[0].allocations:
        if not isinstance(alloc, mybir.MemoryLocationSet):
            continue
        name = alloc.memorylocations[0].name
        if alloc.kind == "ExternalInput":
            if name != partition_name:
                in_names.append(name)
        elif alloc.kind == "ExternalOutput":
            out_names.append(name)
            shape = tuple(alloc.tensor_shape)
            dtype = mybir.dt.np(alloc.dtype)
            out_avals.append(jax.core.ShapedArray(shape, dtype))
            zero_outs.append(_np.zeros(shape, dtype))
    n_params = len(in_names)
    n_outs = len(out_avals)
    all_in = list(in_names) + list(out_names)
    if partition_name is not None:
        all_in.append(partition_name)

    def _body(*args):
        operands = list(args)
        if partition_name is not None:
            operands.append(bass2jax.partition_id_tensor())
        return tuple(
            bass2jax._bass_exec_p.bind(
                *operands,
                out_avals=tuple(out_avals),
                in_names=tuple(all_in),
                out_names=tuple(out_names),
                lowering_input_output_aliases=(),
                sim_require_finite=True,
                sim_require_nnan=True,
                nc=nc,
            )
        )

    devices = jax.devices()[:N_CORES]
    mesh = Mesh(_np.asarray(devices), ("core",))
    n_tot = n_params + n_outs
    fn = jax.jit(
        jax.shard_map(
            _body,
            mesh=mesh,
            in_specs=(PartitionSpec("core"),) * n_tot,
            out_specs=(PartitionSpec("core"),) * n_outs,
            check_vma=False,
        ),
        donate_argnums=tuple(range(n_params, n_tot)),
        keep_unused=True,
    )
    sharding = NamedSharding(mesh, PartitionSpec("core"))

    def run(concat_inputs_by_name):
        dev_in = [
            jax.device_put(concat_inputs_by_name[nm], sharding) for nm in in_names
        ]
        zs = [
            jax.device_put(
                _np.zeros((N_CORES * z.shape[0],) + z.shape[1:], z.dtype), sharding
            )
            for z in zero_outs
        ]
        outs = fn(*dev_in, *zs)
        return {
            name: _np.asarray(outs[i]).reshape(N_CORES, *out_avals[i].shape)
            for i, name in enumerate(out_names)
        }

    return run


def _host_pack(o):
    """Permute rows to [h2, wp, hp, dp, w2] chunks and split per-tile by MIX."""
    import ml_dtypes

    v = np.ascontiguousarray(o, dtype=np.float32).reshape(
        N, C, PD, 2, 32, 2, 32, 2
    )  # n c pd dp h2 hp w2 wp
    v = v.transpose(0, 1, 2, 4, 7, 5, 3, 6)  # n c pd h2 wp hp dp w2
    rows = v.reshape(N_CORES, N_TILES, P, NG * GW)
    xf_parts, xb_parts = [], []
    for t in range(N_TILES):
        nb, nal, ngam = MIX[t]
        nf = nal + ngam
        blk = rows[:, t]  # [cores, P, NG*GW]
        if nf:
            xf_parts.append(
                blk[:, :, : nf * GW].astype(ml_dtypes.float8_e4m3).reshape(N_CORES, -1)
            )
        xb_parts.append(
            blk[:, :, nf * GW :].astype(ml_dtypes.bfloat16).reshape(N_CORES, -1)
        )
    xf = np.ascontiguousarray(np.concatenate(xf_parts, axis=1))
    xb = np.ascontiguousarray(np.concatenate(xb_parts, axis=1))
    return xf, xb


def kernel(o: np.ndarray, bias: np.ndarray) -> np.ndarray:
    global _NC_CACHE, _RUNNER_CACHE

    if _NC_CACHE is None:
        _NC_CACHE = _build_nc()
    nc = _NC_CACHE

    xf, xb = _host_pack(o)
    b2 = np.ascontiguousarray(bias, dtype=np.float32).reshape(1, C)
    b_rep = np.ascontiguousarray(
        np.broadcast_to(b2, (N_CORES, C)).reshape(N_CORES * 1, C)
    )

    try:
        if _RUNNER_CACHE is None:
            _RUNNER_CACHE = _build_runner(nc)
        res = _RUNNER_CACHE({"xf": xf, "xb": xb, "bias": b_rep})
        out = res["out"].reshape(N_CORES * N_PER_CORE)
    except Exception:
        from concourse.bass_utils import run_bass_kernel_spmd

        in_maps = [
            {"xf": xf[k : k + 1], "xb": xb[k : k + 1], "bias": b2}
            for k in range(N_CORES)
        ]
        r = run_bass_kernel_spmd(nc, in_maps, core_ids=list(range(N_CORES)))
        out = np.concatenate(
            [r.results[k]["out"].reshape(N_PER_CORE) for k in range(N_CORES)]
        )
    return out.reshape(N, 1, 1, 1).astype(np.float32)
